# revision 4
# baseline (speedup 1.0000x reference)
"""Trainium2 Bass kernel for nn_MultiHeadAttention_45749991637228.

Reference computation (torch-source quirks preserved):
  K = x @ Wk^T, Q = x @ Wq^T              (Wv unused)
  S[b,h,i,j] = K_i . Q_j  (no 1/sqrt(d) scale)
  P = softmax_j(S)
  out[b,h,i,:] = sum_j P[i,j] K[j,:]      (keys, not values)
  return out.reshape(b, n, h*hd)          (direct reshape, no transpose)

Sharding: tensor-parallel over the 16 heads -> 2 heads per core (8 cores).
Each core gets full x plus its 128-row slice of Wk/Wq.

Per-core dataflow (all matmuls contract over SBUF partitions):
  xT tiles via PE transpose-mode; Kt/Qt = W_h @ x^T directly in
  [c=2*64 heads, m] transposed layout; scores St[j,i] computed per
  (b,h) with both heads packed into the 128x128 PE array (K=64 each,
  tile_position rows 0-63 / 64-127); exp on ScalarE straight out of
  PSUM (scores are bounded, no max pass needed); numerator AND softmax
  denominator in one matmul with a ones-augmented K_nat stationary
  operand -> numT[65, i]; PE-transpose back, multiply by reciprocal
  of the denominator column, DMA out.
"""

import numpy as np

import concourse.bass as bass
import concourse.mybir as mybir
import concourse.tile as tile
from concourse import bacc
from concourse import bass_utils
from concourse.masks import make_identity

N_CORES = 8
B = 4
N_SEQ = 2048
D = 1024
H_TOTAL = 16
HD = 64
H_LOC = H_TOTAL // N_CORES      # 2 heads per core
C = H_LOC * HD                  # 128 projection cols per core
M_TOT = B * N_SEQ               # 8192 rows of x

F32 = mybir.dt.float32

# ---- perf knobs -------------------------------------------------------
# Storage dtype for matmul operand tiles (xT, Kt, Qt, K_aug, P).
P_DT = F32
# Bitcast f32 matmul operands to float32r (1 cyc/row at N>=256 vs 4).
MM_R = False
# Identity dtype for PE transposes (cost model keys on moving operand).
ID_R = False


def _mm_ap(ap):
    if MM_R and ap.dtype == F32:
        return ap.bitcast(mybir.dt.float32r)
    return ap


def _id_ap(ap):
    if ID_R and ap.dtype == F32:
        return ap.bitcast(mybir.dt.float32r)
    return ap


def build_kernel(nb=B, nseq=N_SEQ):
    """Build the per-core Bass program. nb/nseq shrinkable for fast sim."""
    m_tot = nb * nseq
    n_mc = m_tot // 512          # m-chunks of 512 rows
    n_jc = nseq // 128           # j-chunks per (b,h)
    n_ib = nseq // 512           # i-blocks per (b,h)

    nc = bacc.Bacc("TRN2", target_bir_lowering=False, debug=False,
                   num_devices=N_CORES)
    x = nc.dram_tensor("x", [m_tot, D], F32, kind="ExternalInput")
    wk = nc.dram_tensor("wk", [C, D], F32, kind="ExternalInput")
    wq = nc.dram_tensor("wq", [C, D], F32, kind="ExternalInput")
    out = nc.dram_tensor("out", [nb, H_LOC, nseq, HD], F32,
                         kind="ExternalOutput")

    with tile.TileContext(nc) as tc:
        _build(nc, tc, x, wk, wq, out, nb, nseq, n_mc, n_jc, n_ib)
    nc.compile()
    return nc


def _build(nc, tc, x, wk, wq, out, nb, nseq, n_mc, n_jc, n_ib):
    exp = mybir.ActivationFunctionType.Exp

    with (
        tc.tile_pool(name="const", bufs=1) as const_pool,
        tc.tile_pool(name="persist", bufs=1) as persist,
        tc.tile_pool(name="small_ps", bufs=2, space="PSUM") as small_ps,
    ):
        ident = const_pool.tile([128, 128], F32)
        make_identity(nc, ident)

        # ---------------- phase 0: weight transposes ----------------
        # wk/wq rows are the projection output cols c; we need
        # W^T chunks [d 128, c 128] as stationary operands.
        wkT = persist.tile([128, D], P_DT)
        wqT = persist.tile([128, D], P_DT)
        with tc.tile_pool(name="wload", bufs=2) as wload:
            for w_in, w_t in ((wk, wkT), (wq, wqT)):
                wtile = wload.tile([128, D], F32, name="wtile")
                nc.sync.dma_start(wtile[:], w_in[:, :])
                for dc in range(8):
                    ps = small_ps.tile([128, 128], F32, name="ps_tr", tag="small")
                    nc.tensor.matmul(ps[:], wtile[:, bass.ts(dc, 128)],
                                     _id_ap(ident[:]), is_transpose=True,
                                     start=True, stop=True)
                    nc.vector.tensor_copy(w_t[:, bass.ts(dc, 128)], ps[:])

        # Kt/Qt: [c 128, m] transposed projections, persistent.
        kt2 = persist.tile([128, nb * nseq], P_DT)
        qt2 = persist.tile([128, nb * nseq], P_DT)

        # ---------------- phase 1: x^T and projections ----------------
        with (
            tc.tile_pool(name="xload", bufs=2) as xload,
            tc.tile_pool(name="xt_sb", bufs=2) as xt_sbp,
            tc.tile_pool(name="xt_ps", bufs=2, space="PSUM") as xt_ps,
            tc.tile_pool(name="kq_ps", bufs=2, space="PSUM") as kq_ps,
        ):
            for mc in range(n_mc):
                # one DMA: 512 rows of x -> [p 128, g 4, d 1024]
                xt_in = xload.tile([128, 4, D], F32, name="xt_in")
                nc.sync.dma_start(
                    xt_in[:],
                    x[mc * 512:(mc + 1) * 512, :].rearrange(
                        "(g p) d -> p g d", p=128))
                # transpose 512 rows x 1024 cols -> xt_sb [d 1024, m 512]
                xt_sb = xt_sbp.tile([128, 8 * 512], P_DT, name="xt_sb")
                for dc in range(8):
                    ps = xt_ps.tile([128, 512], F32, name="ps_xt")
                    for ms in range(4):
                        nc.tensor.matmul(
                            ps[:, bass.ts(ms, 128)],
                            xt_in[:, ms, bass.ts(dc, 128)],
                            _id_ap(ident[:]), is_transpose=True,
                            start=(ms == 0), stop=(ms == 3))
                    nc.vector.tensor_copy(xt_sb[:, bass.ts(dc, 512)], ps[:])
                # projections for this m-chunk
                for w_t, kq in ((wkT, kt2), (wqT, qt2)):
                    ps = kq_ps.tile([128, 512], F32, name="ps_kq")
                    for dc in range(8):
                        nc.tensor.matmul(
                            ps[:],
                            _mm_ap(w_t[:, bass.ts(dc, 128)]),
                            _mm_ap(xt_sb[:, bass.ts(dc, 512)]),
                            start=(dc == 0), stop=(dc == 7))
                    nc.vector.tensor_copy(kq[:, bass.ts(mc, 512)], ps[:])

        # ---------------- phase 2: attention per (b, head-pair) -------
        with (
            tc.tile_pool(name="kaug_sb", bufs=2) as kaug_sbp,
            tc.tile_pool(name="p_sb", bufs=4) as p_sbp,
            tc.tile_pool(name="numt_sb", bufs=2) as numt_sbp,
            tc.tile_pool(name="o_sb", bufs=4) as o_sbp,
            tc.tile_pool(name="r_sb", bufs=4) as r_sbp,
            tc.tile_pool(name="s_ps", bufs=4, space="PSUM") as s_ps,
            tc.tile_pool(name="num_ps", bufs=2, space="PSUM") as num_ps,
        ):
            for b in range(nb):
                # K_nat tiles [j 128, 64] + ones col -> kaug [128, 65/tile]
                kaug = kaug_sbp.tile([128, H_LOC * n_jc * 65], P_DT,
                                     name="kaug")
                for h in range(H_LOC):
                    for jc in range(n_jc):
                        ps = small_ps.tile([128, 64], F32, name="ps_kn", tag="small")
                        nc.tensor.matmul(
                            ps[:],
                            kt2[h * 64:(h + 1) * 64,
                                b * nseq + jc * 128: b * nseq + (jc + 1) * 128],
                            _id_ap(ident[h * 64:(h + 1) * 64,
                                         h * 64:(h + 1) * 64]),
                            is_transpose=True, start=True, stop=True)
                        t = h * n_jc + jc
                        nc.vector.tensor_copy(kaug[:, t * 65: t * 65 + 64],
                                              ps[:])
                # ones columns (col 64 of each 65-wide tile), one memset
                ones_view = kaug.rearrange("p (t c) -> p t c", c=65)[:, :, 64]
                nc.vector.memset(ones_view, 1.0)

                for ib in range(n_ib):
                    nums = [num_ps.tile([65, 512], F32, name="num")
                            for _ in range(H_LOC)]
                    i0 = b * nseq + ib * 512
                    for jc in range(n_jc):
                        j0 = b * nseq + jc * 128
                        for h in range(H_LOC):
                            s_psum = s_ps.tile([128, 512], F32, name="s_psum")
                            nc.tensor.matmul(
                                s_psum[:],
                                _mm_ap(qt2[h * 64:(h + 1) * 64, j0:j0 + 128]),
                                _mm_ap(kt2[h * 64:(h + 1) * 64, i0:i0 + 512]),
                                start=True, stop=True,
                                tile_position=(h * 64, 0))
                            p_sb = p_sbp.tile([128, 512], P_DT, name="p_sb")
                            nc.scalar.activation(p_sb[:], s_psum[:], exp)
                            t = h * n_jc + jc
                            nc.tensor.matmul(
                                nums[h][:],
                                _mm_ap(kaug[:, t * 65:(t + 1) * 65]),
                                _mm_ap(p_sb[:]),
                                start=(jc == 0), stop=(jc == n_jc - 1))
                    for h in range(H_LOC):
                        numt = numt_sbp.tile([65, 512], F32, name="numt")
                        nc.vector.tensor_copy(numt[:], nums[h][:])
                        for t in range(4):
                            ps = small_ps.tile([128, 65], F32, name="ps_o", tag="small")
                            nc.tensor.matmul(
                                ps[:], numt[:, bass.ts(t, 128)],
                                _id_ap(ident[:65, :65]),
                                is_transpose=True, start=True, stop=True)
                            recip = r_sbp.tile([128, 1], F32, name="recip")
                            nc.vector.reciprocal(recip[:], ps[:, 64:65])
                            o_sb = o_sbp.tile([128, HD], F32, name="o_sb")
                            nc.vector.tensor_scalar_mul(o_sb[:], ps[:, 0:64],
                                                        recip[:])
                            r0 = ib * 512 + t * 128
                            nc.sync.dma_start(out[b, h, r0:r0 + 128, :],
                                              o_sb[:])


# ----------------------------------------------------------------------
# host-side entry point
# ----------------------------------------------------------------------
_NC_CACHE = {}


def _get_nc():
    if "nc" not in _NC_CACHE:
        _NC_CACHE["nc"] = build_kernel()
    return _NC_CACHE["nc"]


def make_in_maps(x, Wk, Wq):
    x_flat = np.ascontiguousarray(
        np.asarray(x, dtype=np.float32).reshape(M_TOT, D))
    Wk = np.asarray(Wk, dtype=np.float32)
    Wq = np.asarray(Wq, dtype=np.float32)
    in_maps = []
    for c in range(N_CORES):
        rows = slice(c * C, (c + 1) * C)
        in_maps.append({
            "x": x_flat,
            "wk": np.ascontiguousarray(Wk[rows]),
            "wq": np.ascontiguousarray(Wq[rows]),
        })
    return in_maps


def gather_out(results):
    # per-core out: (B, H_LOC, N_SEQ, HD) -> (B, H_TOTAL, N_SEQ, HD)
    full = np.concatenate([results[c]["out"] for c in range(N_CORES)], axis=1)
    return np.ascontiguousarray(full.reshape(B, N_SEQ, H_TOTAL * HD))


def kernel(x, Wk, Wq, Wv=None, **_unused):
    nc = _get_nc()
    res = bass_utils.run_bass_kernel_spmd(
        nc, make_in_maps(x, Wk, Wq), core_ids=list(range(N_CORES)))
    return gather_out(res.results)
